# revision 6
# baseline (speedup 1.0000x reference)
"""Trainium2 Bass kernel v4: dual-softmax ("contrast") multi-head self-attention.

Problem (per full input):
  x, y: (4, 1024, 1024) f32; Wq/Wk/Wv: (1024, 1024) f32, nh=16 heads, dk=dv=64.
  q = x @ Wq.T, k = x @ Wk.T, v = y @ Wv.T  (split heads)
  dist   = softmax(q k^T / 8);  c_att = softmax(1-dist) @ v;  att = softmax(dist) @ v

Sharding: 8 cores = 4 batches x 2 head-groups (8 heads each). Each core gets
x[b], y[b] and a 512-row slice of each weight; returns (c_att, att) [1024, 512].

v4 schedule (cost-model-guided; ACT's 128 exp tiles ~133us/core are the floor,
so emission order = per-engine execution order is arranged to keep ACT fed):
  - front(h) = S + E1(exp,accum) + r1 + in-place normalize + DMA-transpose of
    dist into dt[h] (HWDGE DmaTransposeAnt, bf16). back(h) = E3=exp(dt),
    E2=1/E3 (DVE recip-approx), O-matmuls, transposed output + divide.
  - emission: wqT wkT xT(h0) qk0ch0 xT(h1) qk0ch1 wvtT f0 qk1(pj) f1 f2 f3
    ytT | V0 V1 qk2 | blocks h=0..7: back(h) [qk3@h1] front(h+4) V(h+2) out(h)
  - PSUM: tp(2x[P,512]) + pj(1x[P,N]) + big ring(2x[P,N], shared by qk0/qk2/
    qk3 projections and S tiles) = 16KB in setup; big + pso(o3/o2/tb/vh ring)
    = 16KB after.
  - BIR f32r rule: anything consumed as f32r by a matmul must be written by a
    f32r-rounded producer (vv, e3); setup transposes stay plain f32.
  - bf16 DRAM outputs (host casts back to f32): halves the store tail.
"""

import sys

if "/opt/trn_rl_repo" not in sys.path:
    sys.path.insert(0, "/opt/trn_rl_repo")

from contextlib import ExitStack

import numpy as np

import concourse.bass as bass
from concourse import bacc, masks, mybir
from concourse.bass_utils import run_bass_kernel_spmd
from concourse.tile import TileContext

F32 = mybir.dt.float32
F32R = mybir.dt.float32r
BF16 = mybir.dt.bfloat16
EXP = mybir.ActivationFunctionType.Exp

P = 128
N = 1024
D = 1024
NF = 512
FH = 8
DK = 64
NPT = N // P
KBN = D // P
MB = NF // P
RAWK = 8


def _r(ap):
    return ap.bitcast(F32R)


def build_nc():
    nc = bacc.Bacc("TRN2")
    x_d = nc.dram_tensor("x", [N, D], F32, kind="ExternalInput")
    y_d = nc.dram_tensor("y", [N, D], F32, kind="ExternalInput")
    wq_d = nc.dram_tensor("wq", [NF, D], F32, kind="ExternalInput")
    wk_d = nc.dram_tensor("wk", [NF, D], F32, kind="ExternalInput")
    wv_d = nc.dram_tensor("wv", [NF, D], F32, kind="ExternalInput")
    catt_d = nc.dram_tensor("catt", [N, NF], BF16, kind="ExternalOutput")
    att_d = nc.dram_tensor("att", [N, NF], BF16, kind="ExternalOutput")

    with TileContext(nc) as tc, ExitStack() as ctx:
        persist = ctx.enter_context(tc.tile_pool(name="persist", bufs=1))
        ident = persist.tile([P, P], F32)
        masks.make_identity(nc, ident[:])

        qt = persist.tile([P, MB, N], BF16)
        kt = persist.tile([P, MB, N], BF16)
        vv = persist.tile([P, NPT, FH, DK + 1], BF16)
        nc.gpsimd.memset(vv[:, :, :, :], 1.0)

        ytp = ctx.enter_context(tc.tile_pool(name="ytp", bufs=1))
        yt = ytp.tile([P, KBN, N], BF16)
        wvt = ytp.tile([P, KBN, NF], BF16)

        e1p = ctx.enter_context(tc.tile_pool(name="e1p", bufs=17))
        dtp = ctx.enter_context(tc.tile_pool(name="dtp", bufs=3))
        smp = ctx.enter_context(tc.tile_pool(name="smp", bufs=32))
        psb = ctx.enter_context(tc.tile_pool(name="psb", bufs=2, space="PSUM"))

        dt_tiles = {}

        def front_head_init(h):
            dt_tiles[h] = dtp.tile([P, KBN, N], BF16, tag="dt", name=f"dt{h}")

        def front_step(h, qb):
            hb, ho = h // 2, (h % 2) * DK
            dtile = dt_tiles[h]
            s_ps = psb.tile([P, N], F32, tag="big", name=f"s{h}_{qb}")
            for ch in range(2):
                nc.tensor.matmul(
                    s_ps[:, ch * 512:(ch + 1) * 512],
                    lhsT=qt[ho:ho + DK, hb, qb * P:(qb + 1) * P],
                    rhs=kt[ho:ho + DK, hb, ch * 512:(ch + 1) * 512],
                    start=True,
                    stop=True,
                )
            e1 = e1p.tile([P, N], BF16, tag="e1", name=f"e1_{h}_{qb}")
            rs1 = smp.tile([P, 1], F32, tag="rs", name=f"rs{h}_{qb}")
            nc.scalar.activation(e1[:], s_ps[:], EXP, scale=0.125,
                                 accum_out=rs1[:])
            r1 = smp.tile([P, 1], F32, tag="r1", name=f"r1_{h}_{qb}")
            nc.vector.reciprocal_approx_fast(r1[:], rs1[:])
            nc.gpsimd.tensor_scalar_mul(e1[:], e1[:], r1[:, 0:1])
            nc.sync.dma_start_transpose(
                dtile[:, 0:KBN, qb * P:(qb + 1) * P], e1[:])

        def front(h):
            front_head_init(h)
            for qb in range(NPT):
                front_step(h, qb)

        # ---------------- setup ----------------
        with ExitStack() as sctx:
            pst = sctx.enter_context(tc.tile_pool(name="pst", bufs=1, space="PSUM"))
            pjp = sctx.enter_context(tc.tile_pool(name="pjp", bufs=1, space="PSUM"))
            rawp = sctx.enter_context(tc.tile_pool(name="rawp", bufs=1))
            sbA = sctx.enter_context(tc.tile_pool(name="sbA", bufs=1))
            xt = sbA.tile([P, KBN, N], BF16)
            wqt = sbA.tile([P, KBN, NF], BF16)
            wkt = sbA.tile([P, KBN, NF], BF16)

            raw_n = [0]

            def raw_tile():
                t = rawp.tile([P, D], F32, tag="raw", bufs=RAWK,
                              name=f"raw{raw_n[0]}")
                raw_n[0] += 1
                return t

            x_raw = [raw_tile() for _ in range(NPT)]
            wq_raw = [raw_tile() for _ in range(MB)]
            wk_raw = [raw_tile() for _ in range(MB)]
            wv_raw = [raw_tile() for _ in range(MB)]
            y_raw = [raw_tile() for _ in range(NPT)]
            for i in range(NPT):
                nc.sync.dma_start(out=x_raw[i][:], in_=x_d[i * P:(i + 1) * P, :])
            # mb0 rows of Wq/Wk first: they gate the first S/E1 chain
            nc.sync.dma_start(out=wq_raw[0][:], in_=wq_d[0:P, :])
            nc.sync.dma_start(out=wk_raw[0][:], in_=wk_d[0:P, :])
            for m in range(1, MB):
                nc.sync.dma_start(out=wq_raw[m][:], in_=wq_d[m * P:(m + 1) * P, :])
            for m in range(1, MB):
                nc.sync.dma_start(out=wk_raw[m][:], in_=wk_d[m * P:(m + 1) * P, :])
            for m in range(MB):
                nc.sync.dma_start(out=wv_raw[m][:], in_=wv_d[m * P:(m + 1) * P, :])
            for i in range(NPT):
                nc.sync.dma_start(out=y_raw[i][:], in_=y_d[i * P:(i + 1) * P, :])

            tp_n = [0]

            def transpose_range(dst, raw_tiles, lo, hi):
                nb = hi - lo
                for kb in range(KBN):
                    tp = pst.tile([P, 512], F32, tag="tp", bufs=2,
                                  name=f"tp{tp_n[0]}")
                    tp_n[0] += 1
                    for j in range(nb):
                        nc.tensor.transpose(
                            tp[:, j * P:(j + 1) * P],
                            raw_tiles[lo + j][:, kb * P:(kb + 1) * P],
                            ident[:],
                        )
                    nc.vector.tensor_copy(
                        dst[:, kb, lo * P:lo * P + nb * P],
                        tp[:, 0:nb * P],
                    )

            def transpose_half(dst, raw_tiles, nblk, half):
                nb = min(4, nblk - half * 4)
                transpose_range(dst, raw_tiles, half * 4, half * 4 + nb)

            def proj_qk_ch(wt, m, ch, q_ps, kb0=0, kb1=KBN):
                for kb in range(kb0, kb1):
                    nc.tensor.matmul(
                        q_ps[:, ch * 512:(ch + 1) * 512],
                        lhsT=wt[:, kb, m * P:(m + 1) * P],
                        rhs=xt[:, kb, ch * 512:(ch + 1) * 512],
                        start=(kb == 0),
                        stop=(kb == KBN - 1),
                    )

            transpose_half(xt, x_raw, NPT, 0)
            transpose_half(xt, x_raw, NPT, 1)
            transpose_range(wqt, wq_raw, 0, 1)
            transpose_range(wkt, wk_raw, 0, 1)

            q0_ps = psb.tile([P, N], F32, tag="big", name="q0ps")
            k0_ps = psb.tile([P, N], F32, tag="big", name="k0ps")
            proj_qk_ch(wqt, 0, 0, q0_ps)
            proj_qk_ch(wqt, 0, 1, q0_ps)
            nc.vector.tensor_copy(qt[:, 0, :], q0_ps[:])
            proj_qk_ch(wkt, 0, 0, k0_ps)
            proj_qk_ch(wkt, 0, 1, k0_ps)
            nc.vector.tensor_copy(kt[:, 0, :], k0_ps[:])

            # --- fillers: PE work interleaved into the ring-gated front
            # steps so no wall of matmuls ever delays the next S/E1 pair ---
            def fill_tr(dst, raws, lo, hi, kb):
                nb = hi - lo
                def f():
                    tp = pst.tile([P, 512], F32, tag="tp", bufs=2,
                                  name=f"tp{tp_n[0]}")
                    tp_n[0] += 1
                    for j in range(nb):
                        nc.tensor.transpose(
                            tp[:, j * P:(j + 1) * P],
                            raws[lo + j][:, kb * P:(kb + 1) * P],
                            ident[:],
                        )
                    nc.vector.tensor_copy(
                        dst[:, kb, lo * P:lo * P + nb * P],
                        tp[:, 0:nb * P],
                    )
                return f

            pj_tiles = {}

            def fill_pj_alloc(nm):
                def f():
                    pj_tiles[nm] = pjp.tile([P, N], F32, tag="pj",
                                            name=f"{nm}ps")
                return f

            def fill_pj_ch(nm, wt, m, ch, kb0=0, kb1=KBN):
                def f():
                    proj_qk_ch(wt, m, ch, pj_tiles[nm], kb0, kb1)
                return f

            def fill_pj_copy(nm, out_sb, m):
                def f():
                    nc.vector.tensor_copy(out_sb[:, m, :], pj_tiles[nm][:])
                return f

            vh_tiles = {}

            def fill_v_alloc(vh):
                def f():
                    vh_tiles[vh] = pst.tile([P, NF], F32, tag="tp", bufs=2,
                                            name=f"vh{vh}ps")
                return f

            def fill_v_mm(vh, i0, i1):
                def f():
                    vh_ps = vh_tiles[vh]
                    for i in range(i0, i1):
                        for kb in range(KBN):
                            nc.tensor.matmul(
                                vh_ps[:, i * DK:(i + 1) * DK],
                                lhsT=yt[:, kb, i * P:(i + 1) * P],
                                rhs=wvt[:, kb, vh * DK:(vh + 1) * DK],
                                start=(kb == 0),
                                stop=(kb == KBN - 1),
                            )
                return f

            def fill_v_copy(vh):
                def f():
                    nc.vector.tensor_copy(
                        vv[:, 0:NPT, vh, 0:DK],
                        vh_tiles[vh][:].rearrange("p (i d) -> p i d", i=NPT),
                    )
                return f

            fillers = []  # (cost_ns, thunk)
            for kb in range(KBN):
                fillers.append((260, fill_tr(wqt, wq_raw, 1, MB, kb)))
            for kb in range(KBN):
                fillers.append((260, fill_tr(wkt, wk_raw, 1, MB, kb)))
            fillers.append((0, fill_pj_alloc("q1")))
            for ch in range(2):
                for k4 in range(2):
                    fillers.append((860, fill_pj_ch("q1", wqt, 1, ch,
                                                    k4 * 4, k4 * 4 + 4)))
            fillers.append((0, fill_pj_copy("q1", qt, 1)))
            fillers.append((0, fill_pj_alloc("k1")))
            for ch in range(2):
                for k4 in range(2):
                    fillers.append((860, fill_pj_ch("k1", wkt, 1, ch,
                                                    k4 * 4, k4 * 4 + 4)))
            fillers.append((0, fill_pj_copy("k1", kt, 1)))
            for kb in range(KBN):
                fillers.append((330, fill_tr(wvt, wv_raw, 0, MB, kb)))
            for kb in range(KBN):
                fillers.append((330, fill_tr(yt, y_raw, 0, 4, kb)))
            for kb in range(KBN):
                fillers.append((330, fill_tr(yt, y_raw, 4, 8, kb)))
            for vh in (0, 1):
                fillers.append((0, fill_v_alloc(vh)))
                for i4 in range(4):
                    fillers.append((440, fill_v_mm(vh, i4 * 2, i4 * 2 + 2)))
                fillers.append((0, fill_v_copy(vh)))
            for nm, wt, out_sb in (("q3", wqt, qt), ("k3", wkt, kt)):
                fillers.append((0, fill_pj_alloc(nm)))
                for ch in range(2):
                    for k4 in range(2):
                        fillers.append((860, fill_pj_ch(nm, wt, 3, ch,
                                                        k4 * 4, k4 * 4 + 4)))
                fillers.append((0, fill_pj_copy(nm, out_sb, 3)))

            fi = [0]

            def drain(budget):
                while fi[0] < len(fillers):
                    cost, f = fillers[fi[0]]
                    if cost > budget and cost > 0:
                        break
                    f()
                    fi[0] += 1
                    budget -= cost

            for h in range(3):
                front_head_init(h)
                for qb in range(NPT):
                    front_step(h, qb)
                    drain(1250)
            while fi[0] < len(fillers):
                fillers[fi[0]][1]()
                fi[0] += 1

            # qk2 via the S ring (gated on E1(3,*) reads, runs in their shadow)
            for wt, out_sb, nm in ((wqt, qt, "q2"), (wkt, kt, "k2")):
                q_ps = psb.tile([P, N], F32, tag="big", name=f"{nm}ps")
                proj_qk_ch(wt, 2, 0, q_ps)
                proj_qk_ch(wt, 2, 1, q_ps)
                nc.vector.tensor_copy(out_sb[:, 2, :], q_ps[:])

        # ---------------- post-setup pools ----------------
        attp = ctx.enter_context(tc.tile_pool(name="attp", bufs=1))
        att_sb = attp.tile([P, NPT, NF], BF16)
        catt_sb = attp.tile([P, NPT, NF], BF16)
        e3p = ctx.enter_context(tc.tile_pool(name="e3p", bufs=10))
        e2p = ctx.enter_context(tc.tile_pool(name="e2p", bufs=10))
        pso = ctx.enter_context(tc.tile_pool(name="pso", bufs=2, space="PSUM"))

        def v_proj(h):
            vh_ps = psb.tile([P, NF], F32, tag="big", name=f"vh{h}")
            for i in range(NPT):
                for kb in range(KBN):
                    nc.tensor.matmul(
                        vh_ps[:, i * DK:(i + 1) * DK],
                        lhsT=yt[:, kb, i * P:(i + 1) * P],
                        rhs=wvt[:, kb, h * DK:(h + 1) * DK],
                        start=(kb == 0),
                        stop=(kb == KBN - 1),
                    )
            nc.vector.tensor_copy(
                vv[:, 0:NPT, h, 0:DK],
                vh_ps[:].rearrange("p (i d) -> p i d", i=NPT),
            )


        from concourse.dve_ops import (
            RECIP_APPROX_FAST_CONSTS,
            RECIPROCAL_APPROX_FAST,
        )
        cc = RECIP_APPROX_FAST_CONSTS

        W65 = DK + 1
        for h in range(FH):
            dtile = dt_tiles[h]
            # flipped orientation: out [q, dv+1] directly (no transposes,
            # no PSUM->SBUF copies; Pool cannot read PSUM on HW)
            o3q = [pso.tile([P, 4 * W65], F32, tag="o", name=f"o3_{h}_{hf}", bufs=4)
                   for hf in range(2)]
            o2q = [pso.tile([P, 4 * W65], F32, tag="o", name=f"o2_{h}_{hf}", bufs=4)
                   for hf in range(2)]
            e3s, e2s = [], []
            for j in range(KBN):
                e3 = e3p.tile([P, N], BF16, tag="e3", name=f"e3_{h}_{j}")
                nc.scalar.activation(e3[:], dtile[:, j, :], EXP)
                e2 = e2p.tile([P, N], BF16, tag="e2", name=f"e2_{h}_{j}")
                with nc.allow_low_precision(reason="bf16 recip; softmax renorm"):
                    nc.vector.reciprocal(e2[:], e3[:])
                e3s.append(e3)
                e2s.append(e2)
            # PSUM accumulation groups are bank-granular: run each output
            # region's 8-step accumulation to completion before the next
            if h + 3 < FH:
                front(h + 3)

            for qb in range(NPT):
                hf, qr = qb // 4, qb % 4
                ob = slice(qr * W65, (qr + 1) * W65)
                for es, oq in ((e3s, o3q), (e2s, o2q)):
                    for j in range(KBN):
                        nc.tensor.matmul(
                            oq[hf][:, ob], lhsT=es[j][:, qb * P:(qb + 1) * P],
                            rhs=vv[:, j, h, :],
                            start=(j == 0), stop=(j == KBN - 1),
                        )
                for bi, (oqs, out_t, out_d) in enumerate(
                        ((o3q, att_sb, att_d), (o2q, catt_sb, catt_d))):
                    oq = oqs[qb // 4]
                    c0 = (qb % 4) * W65
                    rr = smp.tile([P, 1], F32, tag="rr", name=f"rr{h}_{qb}_{bi}")
                    nc.vector.reciprocal_approx_fast(rr[:], oq[:, c0 + DK:c0 + DK + 1])
                    nc.vector.tensor_scalar_mul(
                        out_t[:, qb, h * DK:(h + 1) * DK],
                        oq[:, c0:c0 + DK], rr[:, 0:1]
                    )
                    if h == FH - 1:
                        nc.sync.dma_start(out=out_d[qb * P:(qb + 1) * P, :],
                                          in_=out_t[:, qb, :])

            if h + 2 < FH:
                v_proj(h + 2)

    nc.finalize()
    return nc


_NC_CACHE = {}


def _get_nc():
    if "nc" not in _NC_CACHE:
        _NC_CACHE["nc"] = build_nc()
    return _NC_CACHE["nc"]


def _make_in_maps(x, y, Wq, Wk, Wv):
    x = np.ascontiguousarray(np.asarray(x, dtype=np.float32))
    y = np.ascontiguousarray(np.asarray(y, dtype=np.float32))
    Wq = np.ascontiguousarray(np.asarray(Wq, dtype=np.float32))
    Wk = np.ascontiguousarray(np.asarray(Wk, dtype=np.float32))
    Wv = np.ascontiguousarray(np.asarray(Wv, dtype=np.float32))
    in_maps = []
    for c in range(8):
        b, h0 = c // 2, (c % 2) * 8
        rows = slice(h0 * DK, h0 * DK + NF)
        in_maps.append({
            "x": x[b],
            "y": y[b],
            "wq": np.ascontiguousarray(Wq[rows]),
            "wk": np.ascontiguousarray(Wk[rows]),
            "wv": np.ascontiguousarray(Wv[rows]),
        })
    return in_maps


def run_cores(x, y, Wq, Wk, Wv, trace=False, tmpdir=None):
    nc = _get_nc()
    res = run_bass_kernel_spmd(
        nc, _make_in_maps(x, y, Wq, Wk, Wv), core_ids=list(range(8)),
        trace=trace, tmpdir=tmpdir,
    )
    B = 4
    c_att = np.empty((B, N, 2 * NF), dtype=np.float32)
    att = np.empty((B, N, 2 * NF), dtype=np.float32)
    for c, r in enumerate(res.results):
        b, cols = c // 2, slice((c % 2) * NF, (c % 2) * NF + NF)
        c_att[b][:, cols] = np.asarray(r["catt"]).astype(np.float32)
        att[b][:, cols] = np.asarray(r["att"]).astype(np.float32)
    return (c_att, att), res


def kernel(x, y, Wq, Wk, Wv):
    out, _ = run_cores(x, y, Wq, Wk, Wv)
    return out


# revision 7
# speedup vs baseline: 1.0037x; 1.0037x over previous
"""Trainium2 Bass kernel v4: dual-softmax ("contrast") multi-head self-attention.

Problem (per full input):
  x, y: (4, 1024, 1024) f32; Wq/Wk/Wv: (1024, 1024) f32, nh=16 heads, dk=dv=64.
  q = x @ Wq.T, k = x @ Wk.T, v = y @ Wv.T  (split heads)
  dist   = softmax(q k^T / 8);  c_att = softmax(1-dist) @ v;  att = softmax(dist) @ v

Sharding: 8 cores = 4 batches x 2 head-groups (8 heads each). Each core gets
x[b], y[b] and a 512-row slice of each weight; returns (c_att, att) [1024, 512].

v4 schedule (cost-model-guided; ACT's 128 exp tiles ~133us/core are the floor,
so emission order = per-engine execution order is arranged to keep ACT fed):
  - front(h) = S + E1(exp,accum) + r1 + in-place normalize + DMA-transpose of
    dist into dt[h] (HWDGE DmaTransposeAnt, bf16). back(h) = E3=exp(dt),
    E2=1/E3 (DVE recip-approx), O-matmuls, transposed output + divide.
  - emission: wqT wkT xT(h0) qk0ch0 xT(h1) qk0ch1 wvtT f0 qk1(pj) f1 f2 f3
    ytT | V0 V1 qk2 | blocks h=0..7: back(h) [qk3@h1] front(h+4) V(h+2) out(h)
  - PSUM: tp(2x[P,512]) + pj(1x[P,N]) + big ring(2x[P,N], shared by qk0/qk2/
    qk3 projections and S tiles) = 16KB in setup; big + pso(o3/o2/tb/vh ring)
    = 16KB after.
  - BIR f32r rule: anything consumed as f32r by a matmul must be written by a
    f32r-rounded producer (vv, e3); setup transposes stay plain f32.
  - bf16 DRAM outputs (host casts back to f32): halves the store tail.
"""

import sys

if "/opt/trn_rl_repo" not in sys.path:
    sys.path.insert(0, "/opt/trn_rl_repo")

from contextlib import ExitStack

import numpy as np

import concourse.bass as bass
from concourse import bacc, masks, mybir
from concourse.bass_utils import run_bass_kernel_spmd
from concourse.tile import TileContext

F32 = mybir.dt.float32
F32R = mybir.dt.float32r
BF16 = mybir.dt.bfloat16
EXP = mybir.ActivationFunctionType.Exp

P = 128
N = 1024
D = 1024
NF = 512
FH = 8
DK = 64
NPT = N // P
KBN = D // P
MB = NF // P
RAWK = 8


def _r(ap):
    return ap.bitcast(F32R)


def build_nc():
    nc = bacc.Bacc("TRN2")
    x_d = nc.dram_tensor("x", [N, D], F32, kind="ExternalInput")
    y_d = nc.dram_tensor("y", [N, D], F32, kind="ExternalInput")
    wq_d = nc.dram_tensor("wq", [NF, D], F32, kind="ExternalInput")
    wk_d = nc.dram_tensor("wk", [NF, D], F32, kind="ExternalInput")
    wv_d = nc.dram_tensor("wv", [NF, D], F32, kind="ExternalInput")
    catt_d = nc.dram_tensor("catt", [N, NF], BF16, kind="ExternalOutput")
    att_d = nc.dram_tensor("att", [N, NF], BF16, kind="ExternalOutput")

    with TileContext(nc) as tc, ExitStack() as ctx:
        persist = ctx.enter_context(tc.tile_pool(name="persist", bufs=1))
        ident = persist.tile([P, P], F32)
        masks.make_identity(nc, ident[:])

        qt = persist.tile([P, MB, N], BF16)
        kt = persist.tile([P, MB, N], BF16)
        vv = persist.tile([P, NPT, FH, DK + 1], BF16)
        nc.gpsimd.memset(vv[:, :, :, :], 1.0)

        ytp = ctx.enter_context(tc.tile_pool(name="ytp", bufs=1))
        yt = ytp.tile([P, KBN, N], BF16)
        wvt = ytp.tile([P, KBN, NF], BF16)

        e1p = ctx.enter_context(tc.tile_pool(name="e1p", bufs=19))
        dtp = ctx.enter_context(tc.tile_pool(name="dtp", bufs=3))
        smp = ctx.enter_context(tc.tile_pool(name="smp", bufs=32))
        psb = ctx.enter_context(tc.tile_pool(name="psb", bufs=2, space="PSUM"))

        dt_tiles = {}

        def front_head_init(h):
            dt_tiles[h] = dtp.tile([P, KBN, N], BF16, tag="dt", name=f"dt{h}")

        def front_step(h, qb):
            hb, ho = h // 2, (h % 2) * DK
            dtile = dt_tiles[h]
            s_ps = psb.tile([P, N], F32, tag="big", name=f"s{h}_{qb}")
            for ch in range(2):
                nc.tensor.matmul(
                    s_ps[:, ch * 512:(ch + 1) * 512],
                    lhsT=qt[ho:ho + DK, hb, qb * P:(qb + 1) * P],
                    rhs=kt[ho:ho + DK, hb, ch * 512:(ch + 1) * 512],
                    start=True,
                    stop=True,
                )
            e1 = e1p.tile([P, N], BF16, tag="e1", name=f"e1_{h}_{qb}")
            rs1 = smp.tile([P, 1], F32, tag="rs", name=f"rs{h}_{qb}")
            nc.scalar.activation(e1[:], s_ps[:], EXP, scale=0.125,
                                 accum_out=rs1[:])
            r1 = smp.tile([P, 1], F32, tag="r1", name=f"r1_{h}_{qb}")
            nc.vector.reciprocal_approx_fast(r1[:], rs1[:])
            nc.gpsimd.tensor_scalar_mul(e1[:], e1[:], r1[:, 0:1])
            nc.sync.dma_start_transpose(
                dtile[:, 0:KBN, qb * P:(qb + 1) * P], e1[:])

        def front(h):
            front_head_init(h)
            for qb in range(NPT):
                front_step(h, qb)

        # ---------------- setup ----------------
        with ExitStack() as sctx:
            pst = sctx.enter_context(tc.tile_pool(name="pst", bufs=1, space="PSUM"))
            pjp = sctx.enter_context(tc.tile_pool(name="pjp", bufs=1, space="PSUM"))
            rawp = sctx.enter_context(tc.tile_pool(name="rawp", bufs=1))
            sbA = sctx.enter_context(tc.tile_pool(name="sbA", bufs=1))
            xt = sbA.tile([P, KBN, N], BF16)
            wqt = sbA.tile([P, KBN, NF], BF16)
            wkt = sbA.tile([P, KBN, NF], BF16)

            raw_n = [0]

            def raw_tile():
                t = rawp.tile([P, D], F32, tag="raw", bufs=RAWK,
                              name=f"raw{raw_n[0]}")
                raw_n[0] += 1
                return t

            x_raw = [raw_tile() for _ in range(NPT)]
            wq_raw = [raw_tile() for _ in range(MB)]
            wk_raw = [raw_tile() for _ in range(MB)]
            wv_raw = [raw_tile() for _ in range(MB)]
            y_raw = [raw_tile() for _ in range(NPT)]
            for i in range(NPT):
                nc.sync.dma_start(out=x_raw[i][:], in_=x_d[i * P:(i + 1) * P, :])
            # mb0 rows of Wq/Wk first: they gate the first S/E1 chain
            nc.sync.dma_start(out=wq_raw[0][:], in_=wq_d[0:P, :])
            nc.sync.dma_start(out=wk_raw[0][:], in_=wk_d[0:P, :])
            for m in range(1, MB):
                nc.sync.dma_start(out=wq_raw[m][:], in_=wq_d[m * P:(m + 1) * P, :])
            for m in range(1, MB):
                nc.sync.dma_start(out=wk_raw[m][:], in_=wk_d[m * P:(m + 1) * P, :])
            for m in range(MB):
                nc.sync.dma_start(out=wv_raw[m][:], in_=wv_d[m * P:(m + 1) * P, :])
            for i in range(NPT):
                nc.sync.dma_start(out=y_raw[i][:], in_=y_d[i * P:(i + 1) * P, :])

            tp_n = [0]

            def transpose_range(dst, raw_tiles, lo, hi):
                nb = hi - lo
                for kb in range(KBN):
                    tp = pst.tile([P, 512], F32, tag="tp", bufs=2,
                                  name=f"tp{tp_n[0]}")
                    tp_n[0] += 1
                    for j in range(nb):
                        nc.tensor.transpose(
                            tp[:, j * P:(j + 1) * P],
                            raw_tiles[lo + j][:, kb * P:(kb + 1) * P],
                            ident[:],
                        )
                    nc.vector.tensor_copy(
                        dst[:, kb, lo * P:lo * P + nb * P],
                        tp[:, 0:nb * P],
                    )

            def transpose_half(dst, raw_tiles, nblk, half):
                nb = min(4, nblk - half * 4)
                transpose_range(dst, raw_tiles, half * 4, half * 4 + nb)

            def proj_qk_ch(wt, m, ch, q_ps, kb0=0, kb1=KBN):
                for kb in range(kb0, kb1):
                    nc.tensor.matmul(
                        q_ps[:, ch * 512:(ch + 1) * 512],
                        lhsT=wt[:, kb, m * P:(m + 1) * P],
                        rhs=xt[:, kb, ch * 512:(ch + 1) * 512],
                        start=(kb == 0),
                        stop=(kb == KBN - 1),
                    )

            transpose_half(xt, x_raw, NPT, 0)
            transpose_half(xt, x_raw, NPT, 1)
            transpose_range(wqt, wq_raw, 0, 1)
            transpose_range(wkt, wk_raw, 0, 1)

            q0_ps = psb.tile([P, N], F32, tag="big", name="q0ps")
            k0_ps = psb.tile([P, N], F32, tag="big", name="k0ps")
            proj_qk_ch(wqt, 0, 0, q0_ps)
            proj_qk_ch(wqt, 0, 1, q0_ps)
            nc.vector.tensor_copy(qt[:, 0, :], q0_ps[:])
            proj_qk_ch(wkt, 0, 0, k0_ps)
            proj_qk_ch(wkt, 0, 1, k0_ps)
            nc.vector.tensor_copy(kt[:, 0, :], k0_ps[:])

            # --- fillers: PE work interleaved into the ring-gated front
            # steps so no wall of matmuls ever delays the next S/E1 pair ---
            def fill_tr(dst, raws, lo, hi, kb):
                nb = hi - lo
                def f():
                    tp = pst.tile([P, 512], F32, tag="tp", bufs=2,
                                  name=f"tp{tp_n[0]}")
                    tp_n[0] += 1
                    for j in range(nb):
                        nc.tensor.transpose(
                            tp[:, j * P:(j + 1) * P],
                            raws[lo + j][:, kb * P:(kb + 1) * P],
                            ident[:],
                        )
                    nc.vector.tensor_copy(
                        dst[:, kb, lo * P:lo * P + nb * P],
                        tp[:, 0:nb * P],
                    )
                return f

            pj_tiles = {}

            def fill_pj_alloc(nm):
                def f():
                    pj_tiles[nm] = pjp.tile([P, N], F32, tag="pj",
                                            name=f"{nm}ps")
                return f

            def fill_pj_ch(nm, wt, m, ch, kb0=0, kb1=KBN):
                def f():
                    proj_qk_ch(wt, m, ch, pj_tiles[nm], kb0, kb1)
                return f

            def fill_pj_copy(nm, out_sb, m):
                def f():
                    nc.vector.tensor_copy(out_sb[:, m, :], pj_tiles[nm][:])
                return f

            vh_tiles = {}

            def fill_v_alloc(vh):
                def f():
                    vh_tiles[vh] = pst.tile([P, NF], F32, tag="tp", bufs=2,
                                            name=f"vh{vh}ps")
                return f

            def fill_v_mm(vh, i0, i1):
                def f():
                    vh_ps = vh_tiles[vh]
                    for i in range(i0, i1):
                        for kb in range(KBN):
                            nc.tensor.matmul(
                                vh_ps[:, i * DK:(i + 1) * DK],
                                lhsT=yt[:, kb, i * P:(i + 1) * P],
                                rhs=wvt[:, kb, vh * DK:(vh + 1) * DK],
                                start=(kb == 0),
                                stop=(kb == KBN - 1),
                            )
                return f

            def fill_v_copy(vh):
                def f():
                    nc.vector.tensor_copy(
                        vv[:, 0:NPT, vh, 0:DK],
                        vh_tiles[vh][:].rearrange("p (i d) -> p i d", i=NPT),
                    )
                return f

            fillers = []  # (cost_ns, thunk)
            for kb in range(KBN):
                fillers.append((260, fill_tr(wqt, wq_raw, 1, MB, kb)))
            for kb in range(KBN):
                fillers.append((260, fill_tr(wkt, wk_raw, 1, MB, kb)))
            fillers.append((0, fill_pj_alloc("q1")))
            for ch in range(2):
                for k4 in range(2):
                    fillers.append((860, fill_pj_ch("q1", wqt, 1, ch,
                                                    k4 * 4, k4 * 4 + 4)))
            fillers.append((0, fill_pj_copy("q1", qt, 1)))
            fillers.append((0, fill_pj_alloc("k1")))
            for ch in range(2):
                for k4 in range(2):
                    fillers.append((860, fill_pj_ch("k1", wkt, 1, ch,
                                                    k4 * 4, k4 * 4 + 4)))
            fillers.append((0, fill_pj_copy("k1", kt, 1)))
            for kb in range(KBN):
                fillers.append((330, fill_tr(wvt, wv_raw, 0, MB, kb)))
            for kb in range(KBN):
                fillers.append((330, fill_tr(yt, y_raw, 0, 4, kb)))
            for kb in range(KBN):
                fillers.append((330, fill_tr(yt, y_raw, 4, 8, kb)))
            for vh in (0, 1):
                fillers.append((0, fill_v_alloc(vh)))
                for i4 in range(4):
                    fillers.append((440, fill_v_mm(vh, i4 * 2, i4 * 2 + 2)))
                fillers.append((0, fill_v_copy(vh)))
            for nm, wt, out_sb in (("q3", wqt, qt), ("k3", wkt, kt)):
                fillers.append((0, fill_pj_alloc(nm)))
                for ch in range(2):
                    for k4 in range(2):
                        fillers.append((860, fill_pj_ch(nm, wt, 3, ch,
                                                        k4 * 4, k4 * 4 + 4)))
                fillers.append((0, fill_pj_copy(nm, out_sb, 3)))

            fi = [0]

            def drain(budget):
                while fi[0] < len(fillers):
                    cost, f = fillers[fi[0]]
                    if cost > budget and cost > 0:
                        break
                    f()
                    fi[0] += 1
                    budget -= cost

            for h in range(3):
                front_head_init(h)
                for qb in range(NPT):
                    front_step(h, qb)
                    drain(1250)
            while fi[0] < len(fillers):
                fillers[fi[0]][1]()
                fi[0] += 1

            # qk2 via the S ring (gated on E1(3,*) reads, runs in their shadow)
            for wt, out_sb, nm in ((wqt, qt, "q2"), (wkt, kt, "k2")):
                q_ps = psb.tile([P, N], F32, tag="big", name=f"{nm}ps")
                proj_qk_ch(wt, 2, 0, q_ps)
                proj_qk_ch(wt, 2, 1, q_ps)
                nc.vector.tensor_copy(out_sb[:, 2, :], q_ps[:])

        # ---------------- post-setup pools ----------------
        attp = ctx.enter_context(tc.tile_pool(name="attp", bufs=1))
        att_sb = attp.tile([P, NPT, NF], BF16)
        catt_sb = attp.tile([P, NPT, NF], BF16)
        e3p = ctx.enter_context(tc.tile_pool(name="e3p", bufs=12))
        e2p = ctx.enter_context(tc.tile_pool(name="e2p", bufs=12))
        pso = ctx.enter_context(tc.tile_pool(name="pso", bufs=2, space="PSUM"))

        def v_proj(h):
            vh_ps = psb.tile([P, NF], F32, tag="big", name=f"vh{h}")
            for i in range(NPT):
                for kb in range(KBN):
                    nc.tensor.matmul(
                        vh_ps[:, i * DK:(i + 1) * DK],
                        lhsT=yt[:, kb, i * P:(i + 1) * P],
                        rhs=wvt[:, kb, h * DK:(h + 1) * DK],
                        start=(kb == 0),
                        stop=(kb == KBN - 1),
                    )
            nc.vector.tensor_copy(
                vv[:, 0:NPT, h, 0:DK],
                vh_ps[:].rearrange("p (i d) -> p i d", i=NPT),
            )


        from concourse.dve_ops import (
            RECIP_APPROX_FAST_CONSTS,
            RECIPROCAL_APPROX_FAST,
        )
        cc = RECIP_APPROX_FAST_CONSTS

        W65 = DK + 1
        for h in range(FH):
            dtile = dt_tiles[h]
            # flipped orientation: out [q, dv+1] directly (no transposes,
            # no PSUM->SBUF copies; Pool cannot read PSUM on HW)
            o3q = [pso.tile([P, 4 * W65], F32, tag="o", name=f"o3_{h}_{hf}", bufs=4)
                   for hf in range(2)]
            o2q = [pso.tile([P, 4 * W65], F32, tag="o", name=f"o2_{h}_{hf}", bufs=4)
                   for hf in range(2)]
            e3s, e2s = [], []
            for j in range(KBN):
                e3 = e3p.tile([P, N], BF16, tag="e3", name=f"e3_{h}_{j}")
                nc.scalar.activation(e3[:], dtile[:, j, :], EXP)
                e2 = e2p.tile([P, N], BF16, tag="e2", name=f"e2_{h}_{j}")
                with nc.allow_low_precision(reason="bf16 recip; softmax renorm"):
                    nc.vector.reciprocal(e2[:], e3[:])
                e3s.append(e3)
                e2s.append(e2)
            # PSUM accumulation groups are bank-granular: run each output
            # region's 8-step accumulation to completion before the next
            if h + 3 < FH:
                front(h + 3)

            for qb in range(NPT):
                hf, qr = qb // 4, qb % 4
                ob = slice(qr * W65, (qr + 1) * W65)
                for es, oq in ((e3s, o3q), (e2s, o2q)):
                    for j in range(KBN):
                        nc.tensor.matmul(
                            oq[hf][:, ob], lhsT=es[j][:, qb * P:(qb + 1) * P],
                            rhs=vv[:, j, h, :],
                            start=(j == 0), stop=(j == KBN - 1),
                        )
                for bi, (oqs, out_t, out_d) in enumerate(
                        ((o3q, att_sb, att_d), (o2q, catt_sb, catt_d))):
                    oq = oqs[qb // 4]
                    c0 = (qb % 4) * W65
                    rr = smp.tile([P, 1], F32, tag="rr", name=f"rr{h}_{qb}_{bi}")
                    nc.vector.reciprocal_approx_fast(rr[:], oq[:, c0 + DK:c0 + DK + 1])
                    nc.vector.tensor_scalar_mul(
                        out_t[:, qb, h * DK:(h + 1) * DK],
                        oq[:, c0:c0 + DK], rr[:, 0:1]
                    )
                    if h == FH - 1:
                        nc.sync.dma_start(out=out_d[qb * P:(qb + 1) * P, :],
                                          in_=out_t[:, qb, :])

            if h + 2 < FH:
                v_proj(h + 2)

    nc.finalize()
    return nc


_NC_CACHE = {}


def _get_nc():
    if "nc" not in _NC_CACHE:
        _NC_CACHE["nc"] = build_nc()
    return _NC_CACHE["nc"]


def _make_in_maps(x, y, Wq, Wk, Wv):
    x = np.ascontiguousarray(np.asarray(x, dtype=np.float32))
    y = np.ascontiguousarray(np.asarray(y, dtype=np.float32))
    Wq = np.ascontiguousarray(np.asarray(Wq, dtype=np.float32))
    Wk = np.ascontiguousarray(np.asarray(Wk, dtype=np.float32))
    Wv = np.ascontiguousarray(np.asarray(Wv, dtype=np.float32))
    in_maps = []
    for c in range(8):
        b, h0 = c // 2, (c % 2) * 8
        rows = slice(h0 * DK, h0 * DK + NF)
        in_maps.append({
            "x": x[b],
            "y": y[b],
            "wq": np.ascontiguousarray(Wq[rows]),
            "wk": np.ascontiguousarray(Wk[rows]),
            "wv": np.ascontiguousarray(Wv[rows]),
        })
    return in_maps


def run_cores(x, y, Wq, Wk, Wv, trace=False, tmpdir=None):
    nc = _get_nc()
    res = run_bass_kernel_spmd(
        nc, _make_in_maps(x, y, Wq, Wk, Wv), core_ids=list(range(8)),
        trace=trace, tmpdir=tmpdir,
    )
    B = 4
    c_att = np.empty((B, N, 2 * NF), dtype=np.float32)
    att = np.empty((B, N, 2 * NF), dtype=np.float32)
    for c, r in enumerate(res.results):
        b, cols = c // 2, slice((c % 2) * NF, (c % 2) * NF + NF)
        c_att[b][:, cols] = np.asarray(r["catt"]).astype(np.float32)
        att[b][:, cols] = np.asarray(r["att"]).astype(np.float32)
    return (c_att, att), res


def kernel(x, y, Wq, Wk, Wv):
    out, _ = run_cores(x, y, Wq, Wk, Wv)
    return out


# revision 8
# speedup vs baseline: 1.0201x; 1.0164x over previous
"""Trainium2 Bass kernel v4: dual-softmax ("contrast") multi-head self-attention.

Problem (per full input):
  x, y: (4, 1024, 1024) f32; Wq/Wk/Wv: (1024, 1024) f32, nh=16 heads, dk=dv=64.
  q = x @ Wq.T, k = x @ Wk.T, v = y @ Wv.T  (split heads)
  dist   = softmax(q k^T / 8);  c_att = softmax(1-dist) @ v;  att = softmax(dist) @ v

Sharding: 8 cores = 4 batches x 2 head-groups (8 heads each). Each core gets
x[b], y[b] and a 512-row slice of each weight; returns (c_att, att) [1024, 512].

v4 schedule (cost-model-guided; ACT's 128 exp tiles ~133us/core are the floor,
so emission order = per-engine execution order is arranged to keep ACT fed):
  - front(h) = S + E1(exp,accum) + r1 + in-place normalize + DMA-transpose of
    dist into dt[h] (HWDGE DmaTransposeAnt, bf16). back(h) = E3=exp(dt),
    E2=1/E3 (DVE recip-approx), O-matmuls, transposed output + divide.
  - emission: wqT wkT xT(h0) qk0ch0 xT(h1) qk0ch1 wvtT f0 qk1(pj) f1 f2 f3
    ytT | V0 V1 qk2 | blocks h=0..7: back(h) [qk3@h1] front(h+4) V(h+2) out(h)
  - PSUM: tp(2x[P,512]) + pj(1x[P,N]) + big ring(2x[P,N], shared by qk0/qk2/
    qk3 projections and S tiles) = 16KB in setup; big + pso(o3/o2/tb/vh ring)
    = 16KB after.
  - BIR f32r rule: anything consumed as f32r by a matmul must be written by a
    f32r-rounded producer (vv, e3); setup transposes stay plain f32.
  - bf16 DRAM outputs (host casts back to f32): halves the store tail.
"""

import sys

if "/opt/trn_rl_repo" not in sys.path:
    sys.path.insert(0, "/opt/trn_rl_repo")

from contextlib import ExitStack

import numpy as np

import concourse.bass as bass
from concourse import bacc, masks, mybir
from concourse.bass_utils import run_bass_kernel_spmd
from concourse.tile import TileContext

F32 = mybir.dt.float32
F32R = mybir.dt.float32r
BF16 = mybir.dt.bfloat16
EXP = mybir.ActivationFunctionType.Exp

P = 128
N = 1024
D = 1024
NF = 512
FH = 8
DK = 64
NPT = N // P
KBN = D // P
MB = NF // P
RAWK = 8


def _r(ap):
    return ap.bitcast(F32R)


def build_nc():
    nc = bacc.Bacc("TRN2")
    x_d = nc.dram_tensor("x", [N, D], F32, kind="ExternalInput")
    y_d = nc.dram_tensor("y", [N, D], F32, kind="ExternalInput")
    wq_d = nc.dram_tensor("wq", [NF, D], F32, kind="ExternalInput")
    wk_d = nc.dram_tensor("wk", [NF, D], F32, kind="ExternalInput")
    wv_d = nc.dram_tensor("wv", [NF, D], F32, kind="ExternalInput")
    catt_d = nc.dram_tensor("catt", [N, NF], BF16, kind="ExternalOutput")
    att_d = nc.dram_tensor("att", [N, NF], BF16, kind="ExternalOutput")

    with TileContext(nc) as tc, ExitStack() as ctx:
        persist = ctx.enter_context(tc.tile_pool(name="persist", bufs=1))
        ident = persist.tile([P, P], F32)
        masks.make_identity(nc, ident[:])

        qt = persist.tile([P, MB, N], BF16)
        kt = persist.tile([P, MB, N], BF16)
        vv = persist.tile([P, NPT, FH, DK + 1], BF16)
        nc.gpsimd.memset(vv[:, :, :, :], 1.0)

        ytp = ctx.enter_context(tc.tile_pool(name="ytp", bufs=1))
        yt = ytp.tile([P, KBN, N], BF16)
        wvt = ytp.tile([P, KBN, NF], BF16)

        e1p = ctx.enter_context(tc.tile_pool(name="e1p", bufs=19))
        dtp = ctx.enter_context(tc.tile_pool(name="dtp", bufs=3))
        smp = ctx.enter_context(tc.tile_pool(name="smp", bufs=32))
        psb = ctx.enter_context(tc.tile_pool(name="psb", bufs=2, space="PSUM"))

        dt_tiles = {}

        def front_head_init(h):
            dt_tiles[h] = dtp.tile([P, KBN, N], BF16, tag="dt", name=f"dt{h}")

        def front_step(h, qb):
            hb, ho = h // 2, (h % 2) * DK
            dtile = dt_tiles[h]
            s_ps = psb.tile([P, N], F32, tag="big", name=f"s{h}_{qb}")
            for ch in range(2):
                nc.tensor.matmul(
                    s_ps[:, ch * 512:(ch + 1) * 512],
                    lhsT=qt[ho:ho + DK, hb, qb * P:(qb + 1) * P],
                    rhs=kt[ho:ho + DK, hb, ch * 512:(ch + 1) * 512],
                    start=True,
                    stop=True,
                )
            e1 = e1p.tile([P, N], BF16, tag="e1", name=f"e1_{h}_{qb}")
            rs1 = smp.tile([P, 1], F32, tag="rs", name=f"rs{h}_{qb}")
            nc.scalar.activation(e1[:], s_ps[:], EXP, scale=0.125,
                                 accum_out=rs1[:])
            r1 = smp.tile([P, 1], F32, tag="r1", name=f"r1_{h}_{qb}")
            nc.vector.reciprocal_approx_fast(r1[:], rs1[:])
            nc.gpsimd.tensor_scalar_mul(e1[:], e1[:], r1[:, 0:1])
            nc.sync.dma_start_transpose(
                dtile[:, 0:KBN, qb * P:(qb + 1) * P], e1[:])

        def front(h):
            front_head_init(h)
            for qb in range(NPT):
                front_step(h, qb)

        # ---------------- setup ----------------
        with ExitStack() as sctx:
            pst = sctx.enter_context(tc.tile_pool(name="pst", bufs=1, space="PSUM"))
            pjp = sctx.enter_context(tc.tile_pool(name="pjp", bufs=1, space="PSUM"))
            rawp = sctx.enter_context(tc.tile_pool(name="rawp", bufs=1))
            sbA = sctx.enter_context(tc.tile_pool(name="sbA", bufs=1))
            xt = sbA.tile([P, KBN, N], BF16)
            wqt = sbA.tile([P, KBN, NF], BF16)
            wkt = sbA.tile([P, KBN, NF], BF16)

            raw_n = [0]

            def raw_tile():
                t = rawp.tile([P, D], F32, tag="raw", bufs=RAWK,
                              name=f"raw{raw_n[0]}")
                raw_n[0] += 1
                return t

            x_raw = [raw_tile() for _ in range(NPT)]
            wq_raw = [raw_tile() for _ in range(MB)]
            wk_raw = [raw_tile() for _ in range(MB)]
            wv_raw = [raw_tile() for _ in range(MB)]
            y_raw = [raw_tile() for _ in range(NPT)]
            for i in range(NPT):
                nc.sync.dma_start(out=x_raw[i][:], in_=x_d[i * P:(i + 1) * P, :])
            # mb0 rows of Wq/Wk first: they gate the first S/E1 chain
            nc.sync.dma_start(out=wq_raw[0][:], in_=wq_d[0:P, :])
            nc.sync.dma_start(out=wk_raw[0][:], in_=wk_d[0:P, :])
            for m in range(1, MB):
                nc.sync.dma_start(out=wq_raw[m][:], in_=wq_d[m * P:(m + 1) * P, :])
            for m in range(1, MB):
                nc.sync.dma_start(out=wk_raw[m][:], in_=wk_d[m * P:(m + 1) * P, :])
            for m in range(MB):
                nc.sync.dma_start(out=wv_raw[m][:], in_=wv_d[m * P:(m + 1) * P, :])
            for i in range(NPT):
                nc.sync.dma_start(out=y_raw[i][:], in_=y_d[i * P:(i + 1) * P, :])

            tp_n = [0]

            def transpose_range(dst, raw_tiles, lo, hi):
                nb = hi - lo
                for kb in range(KBN):
                    tp = pst.tile([P, 512], F32, tag="tp", bufs=2,
                                  name=f"tp{tp_n[0]}")
                    tp_n[0] += 1
                    for j in range(nb):
                        nc.tensor.transpose(
                            tp[:, j * P:(j + 1) * P],
                            raw_tiles[lo + j][:, kb * P:(kb + 1) * P],
                            ident[:],
                        )
                    nc.vector.tensor_copy(
                        dst[:, kb, lo * P:lo * P + nb * P],
                        tp[:, 0:nb * P],
                    )

            def transpose_half(dst, raw_tiles, nblk, half):
                nb = min(4, nblk - half * 4)
                transpose_range(dst, raw_tiles, half * 4, half * 4 + nb)

            def proj_qk_ch(wt, m, ch, q_ps, kb0=0, kb1=KBN):
                for kb in range(kb0, kb1):
                    nc.tensor.matmul(
                        q_ps[:, ch * 512:(ch + 1) * 512],
                        lhsT=wt[:, kb, m * P:(m + 1) * P],
                        rhs=xt[:, kb, ch * 512:(ch + 1) * 512],
                        start=(kb == 0),
                        stop=(kb == KBN - 1),
                    )

            transpose_half(xt, x_raw, NPT, 0)
            transpose_half(xt, x_raw, NPT, 1)
            transpose_range(wqt, wq_raw, 0, 1)
            transpose_range(wkt, wk_raw, 0, 1)

            q0_ps = psb.tile([P, N], F32, tag="big", name="q0ps")
            k0_ps = psb.tile([P, N], F32, tag="big", name="k0ps")
            proj_qk_ch(wqt, 0, 0, q0_ps)
            proj_qk_ch(wqt, 0, 1, q0_ps)
            nc.vector.tensor_copy(qt[:, 0, :], q0_ps[:])
            proj_qk_ch(wkt, 0, 0, k0_ps)
            proj_qk_ch(wkt, 0, 1, k0_ps)
            nc.vector.tensor_copy(kt[:, 0, :], k0_ps[:])

            # --- fillers: PE work interleaved into the ring-gated front
            # steps so no wall of matmuls ever delays the next S/E1 pair ---
            def fill_tr(dst, raws, lo, hi, kb):
                nb = hi - lo
                def f():
                    tp = pst.tile([P, 512], F32, tag="tp", bufs=2,
                                  name=f"tp{tp_n[0]}")
                    tp_n[0] += 1
                    for j in range(nb):
                        nc.tensor.transpose(
                            tp[:, j * P:(j + 1) * P],
                            raws[lo + j][:, kb * P:(kb + 1) * P],
                            ident[:],
                        )
                    nc.vector.tensor_copy(
                        dst[:, kb, lo * P:lo * P + nb * P],
                        tp[:, 0:nb * P],
                    )
                return f

            pj_tiles = {}

            def fill_pj_alloc(nm):
                def f():
                    pj_tiles[nm] = pjp.tile([P, N], F32, tag="pj",
                                            name=f"{nm}ps")
                return f

            def fill_pj_ch(nm, wt, m, ch, kb0=0, kb1=KBN):
                def f():
                    proj_qk_ch(wt, m, ch, pj_tiles[nm], kb0, kb1)
                return f

            def fill_pj_copy(nm, out_sb, m):
                def f():
                    nc.vector.tensor_copy(out_sb[:, m, :], pj_tiles[nm][:])
                return f

            vh_tiles = {}

            def fill_v_alloc(vh):
                def f():
                    vh_tiles[vh] = pst.tile([P, NF], F32, tag="tp", bufs=2,
                                            name=f"vh{vh}ps")
                return f

            def fill_v_mm(vh, i0, i1):
                def f():
                    vh_ps = vh_tiles[vh]
                    for i in range(i0, i1):
                        for kb in range(KBN):
                            nc.tensor.matmul(
                                vh_ps[:, i * DK:(i + 1) * DK],
                                lhsT=yt[:, kb, i * P:(i + 1) * P],
                                rhs=wvt[:, kb, vh * DK:(vh + 1) * DK],
                                start=(kb == 0),
                                stop=(kb == KBN - 1),
                            )
                return f

            def fill_v_copy(vh):
                def f():
                    nc.vector.tensor_copy(
                        vv[:, 0:NPT, vh, 0:DK],
                        vh_tiles[vh][:].rearrange("p (i d) -> p i d", i=NPT),
                    )
                return f

            fillers = []  # (cost_ns, thunk)
            for kb in range(KBN):
                fillers.append((260, fill_tr(wqt, wq_raw, 1, MB, kb)))
            for kb in range(KBN):
                fillers.append((260, fill_tr(wkt, wk_raw, 1, MB, kb)))
            fillers.append((0, fill_pj_alloc("q1")))
            for ch in range(2):
                for k4 in range(2):
                    fillers.append((860, fill_pj_ch("q1", wqt, 1, ch,
                                                    k4 * 4, k4 * 4 + 4)))
            fillers.append((0, fill_pj_copy("q1", qt, 1)))
            fillers.append((0, fill_pj_alloc("k1")))
            for ch in range(2):
                for k4 in range(2):
                    fillers.append((860, fill_pj_ch("k1", wkt, 1, ch,
                                                    k4 * 4, k4 * 4 + 4)))
            fillers.append((0, fill_pj_copy("k1", kt, 1)))
            for kb in range(KBN):
                fillers.append((330, fill_tr(wvt, wv_raw, 0, MB, kb)))
            for kb in range(KBN):
                fillers.append((330, fill_tr(yt, y_raw, 0, 4, kb)))
            for kb in range(KBN):
                fillers.append((330, fill_tr(yt, y_raw, 4, 8, kb)))
            for vh in (0, 1):
                fillers.append((0, fill_v_alloc(vh)))
                for i4 in range(4):
                    fillers.append((440, fill_v_mm(vh, i4 * 2, i4 * 2 + 2)))
                fillers.append((0, fill_v_copy(vh)))
            for nm, wt, out_sb in (("q3", wqt, qt), ("k3", wkt, kt)):
                fillers.append((0, fill_pj_alloc(nm)))
                for ch in range(2):
                    for k4 in range(2):
                        fillers.append((860, fill_pj_ch(nm, wt, 3, ch,
                                                        k4 * 4, k4 * 4 + 4)))
                fillers.append((0, fill_pj_copy(nm, out_sb, 3)))

            fi = [0]

            def drain(budget):
                while fi[0] < len(fillers):
                    cost, f = fillers[fi[0]]
                    if cost > budget and cost > 0:
                        break
                    f()
                    fi[0] += 1
                    budget -= cost

            for h in range(3):
                front_head_init(h)
                for qb in range(NPT):
                    front_step(h, qb)
                    drain(1250)
            while fi[0] < len(fillers):
                fillers[fi[0]][1]()
                fi[0] += 1

            # qk2 via the pj slab: no S-ring gate, starts right after fillers
            for wt, out_sb, nm in ((wqt, qt, "q2"), (wkt, kt, "k2")):
                q_ps = pjp.tile([P, N], F32, tag="pj", name=f"{nm}ps")
                proj_qk_ch(wt, 2, 0, q_ps)
                proj_qk_ch(wt, 2, 1, q_ps)
                nc.vector.tensor_copy(out_sb[:, 2, :], q_ps[:])

        # ---------------- post-setup pools ----------------
        attp = ctx.enter_context(tc.tile_pool(name="attp", bufs=1))
        att_sb = attp.tile([P, NPT, NF], BF16)
        catt_sb = attp.tile([P, NPT, NF], BF16)
        e3p = ctx.enter_context(tc.tile_pool(name="e3p", bufs=12))
        e2p = ctx.enter_context(tc.tile_pool(name="e2p", bufs=12))
        pso = ctx.enter_context(tc.tile_pool(name="pso", bufs=2, space="PSUM"))

        def v_proj(h):
            vh_ps = psb.tile([P, NF], F32, tag="big", name=f"vh{h}")
            for i in range(NPT):
                for kb in range(KBN):
                    nc.tensor.matmul(
                        vh_ps[:, i * DK:(i + 1) * DK],
                        lhsT=yt[:, kb, i * P:(i + 1) * P],
                        rhs=wvt[:, kb, h * DK:(h + 1) * DK],
                        start=(kb == 0),
                        stop=(kb == KBN - 1),
                    )
            nc.vector.tensor_copy(
                vv[:, 0:NPT, h, 0:DK],
                vh_ps[:].rearrange("p (i d) -> p i d", i=NPT),
            )


        from concourse.dve_ops import (
            RECIP_APPROX_FAST_CONSTS,
            RECIPROCAL_APPROX_FAST,
        )
        cc = RECIP_APPROX_FAST_CONSTS

        W65 = DK + 1
        for h in range(FH):
            dtile = dt_tiles[h]
            # flipped orientation: out [q, dv+1] directly (no transposes,
            # no PSUM->SBUF copies; Pool cannot read PSUM on HW)
            o3q = [pso.tile([P, 4 * W65], F32, tag="o", name=f"o3_{h}_{hf}", bufs=4)
                   for hf in range(2)]
            o2q = [pso.tile([P, 4 * W65], F32, tag="o", name=f"o2_{h}_{hf}", bufs=4)
                   for hf in range(2)]
            e3s, e2s = [], []
            for j in range(KBN):
                e3 = e3p.tile([P, N], BF16, tag="e3", name=f"e3_{h}_{j}")
                nc.scalar.activation(e3[:], dtile[:, j, :], EXP)
                e2 = e2p.tile([P, N], BF16, tag="e2", name=f"e2_{h}_{j}")
                with nc.allow_low_precision(reason="bf16 recip; softmax renorm"):
                    nc.vector.reciprocal(e2[:], e3[:])
                e3s.append(e3)
                e2s.append(e2)
            # PSUM accumulation groups are bank-granular: run each output
            # region's 8-step accumulation to completion before the next
            if h + 3 < FH:
                front(h + 3)

            for qb in range(NPT):
                hf, qr = qb // 4, qb % 4
                ob = slice(qr * W65, (qr + 1) * W65)
                for es, oq in ((e3s, o3q), (e2s, o2q)):
                    for j in range(KBN):
                        nc.tensor.matmul(
                            oq[hf][:, ob], lhsT=es[j][:, qb * P:(qb + 1) * P],
                            rhs=vv[:, j, h, :],
                            start=(j == 0), stop=(j == KBN - 1),
                        )
                for bi, (oqs, out_t, out_d) in enumerate(
                        ((o3q, att_sb, att_d), (o2q, catt_sb, catt_d))):
                    oq = oqs[qb // 4]
                    c0 = (qb % 4) * W65
                    rr = smp.tile([P, 1], F32, tag="rr", name=f"rr{h}_{qb}_{bi}")
                    nc.vector.reciprocal_approx_fast(rr[:], oq[:, c0 + DK:c0 + DK + 1])
                    nc.vector.tensor_scalar_mul(
                        out_t[:, qb, h * DK:(h + 1) * DK],
                        oq[:, c0:c0 + DK], rr[:, 0:1]
                    )
                    if h == FH - 1:
                        nc.sync.dma_start(out=out_d[qb * P:(qb + 1) * P, :],
                                          in_=out_t[:, qb, :])

            if h + 2 < FH:
                v_proj(h + 2)

    nc.finalize()
    return nc


_NC_CACHE = {}


def _get_nc():
    if "nc" not in _NC_CACHE:
        _NC_CACHE["nc"] = build_nc()
    return _NC_CACHE["nc"]


def _make_in_maps(x, y, Wq, Wk, Wv):
    x = np.ascontiguousarray(np.asarray(x, dtype=np.float32))
    y = np.ascontiguousarray(np.asarray(y, dtype=np.float32))
    Wq = np.ascontiguousarray(np.asarray(Wq, dtype=np.float32))
    Wk = np.ascontiguousarray(np.asarray(Wk, dtype=np.float32))
    Wv = np.ascontiguousarray(np.asarray(Wv, dtype=np.float32))
    in_maps = []
    for c in range(8):
        b, h0 = c // 2, (c % 2) * 8
        rows = slice(h0 * DK, h0 * DK + NF)
        in_maps.append({
            "x": x[b],
            "y": y[b],
            "wq": np.ascontiguousarray(Wq[rows]),
            "wk": np.ascontiguousarray(Wk[rows]),
            "wv": np.ascontiguousarray(Wv[rows]),
        })
    return in_maps


def run_cores(x, y, Wq, Wk, Wv, trace=False, tmpdir=None):
    nc = _get_nc()
    res = run_bass_kernel_spmd(
        nc, _make_in_maps(x, y, Wq, Wk, Wv), core_ids=list(range(8)),
        trace=trace, tmpdir=tmpdir,
    )
    B = 4
    c_att = np.empty((B, N, 2 * NF), dtype=np.float32)
    att = np.empty((B, N, 2 * NF), dtype=np.float32)
    for c, r in enumerate(res.results):
        b, cols = c // 2, slice((c % 2) * NF, (c % 2) * NF + NF)
        c_att[b][:, cols] = np.asarray(r["catt"]).astype(np.float32)
        att[b][:, cols] = np.asarray(r["att"]).astype(np.float32)
    return (c_att, att), res


def kernel(x, y, Wq, Wk, Wv):
    out, _ = run_cores(x, y, Wq, Wk, Wv)
    return out
